# revision 1
# baseline (speedup 1.0000x reference)
"""Trainium2 Bass kernel for the 2-layer GraphConv + mean-pool network.

Self-contained: kernel(**inputs) -> np.ndarray [N_GRAPHS] float32.

Strategy (8 NeuronCores, SPMD, 2 launches):
  Launch 1: per-core node shard, compute x @ [W1_rel | W1_root] (+b1 via a
    ones-row) on the tensor engine -> per-node 16-wide projections.
    This exploits segment_sum(x[src]) @ W == segment_sum((x@W)[src]) to cut
    edge traffic 4x.
  Host: expand (x@W1_rel)[src] per edge into a degree-class-sorted,
    partition-aligned layout (pure index/permutation work), replicating the
    halo exchange. Nodes are sharded by contiguous dst ranges.
  Launch 2: per-core, stream edge messages sequentially; segmented reduction
    over each node's incident edges via log-halving strided vector adds
    (bf16 in, f32 accumulate); h = relu(agg + x@W1_root + b1); then a single
    PSUM-accumulated matmul P = [C|B]^T @ h where C[j,g] = #out-edges of node
    j landing in graph g and B = one-hot(batch). This collapses the entire
    second GraphConv layer + global mean pool into one matmul because the
    final output only needs per-graph sums (linearity of layer 2).
  Host: out[g] = (P[g]@(W2_rel@Wlin) + P[64+g]@(W2_root@Wlin)) / count_g
                 + b2@Wlin + blin   (a 64-element finish).
"""
import sys

if "/opt/trn_rl_repo" not in sys.path:
    sys.path.insert(0, "/opt/trn_rl_repo")

import numpy as np
import ml_dtypes

import concourse.bass as bass
import concourse.mybir as mybir
import concourse.tile as tile
from concourse.vector_clock import ScopedClock
from concourse.bass_utils import run_bass_kernel_spmd

N_CORES = 8
P = 128
F = 16
WMAX = 16
N_GRAPHS = 64

# ----------------------------------------------------------------------------
# toolchain workarounds
# ----------------------------------------------------------------------------
_PATCHED = False


def _patch_tile():
    """Walrus (neuronxcc) rejects >~2 sync waits on one instruction; Tile's
    final drain can carry many. Emit them as separate nops instead."""
    global _PATCHED
    if _PATCHED:
        return
    _PATCHED = True

    def patched(self, tick_clock, wait_clock):
        nop = self.nc.sync.nop(nofuse=True)
        wait_clock.add_sem_waits(nop.ins, ScopedClock({None: tick_clock.global_clock}))
        si = nop.ins.sync_info
        if si is not None and si.on_wait and len(si.on_wait) > 1:
            waits = list(si.on_wait)
            si.on_wait = waits[:1]
            for w in waits[1:]:
                nop2 = self.nc.sync.nop(nofuse=True)
                si2 = nop2.ins.sync_info
                if si2 is None:
                    nop2.ins.sync_info = mybir.SyncInfo(on_wait=[w], on_update=[])
                else:
                    si2.on_wait = [w]
        self.nc.sync.drain()
        self.nc.all_engine_barrier()
        assert self.sems is not None
        popped = self.nc._tile_sem_poison_stack.pop()
        assert popped is self._sem_poison
        self.nc.clear_and_free_semaphores(list(self.sems.allocated().values()))
        self.nc.all_engine_barrier()

    tile.TileContext._drain_and_barrier = patched


def _split_sync_waits(nc, max_waits=1):
    """Move excess per-instruction sync waits onto injected NoOps."""
    import bass_rust

    ctr = 0
    for fn in nc.m.functions:
        for bb in fn.blocks:
            insts = list(bb.instructions)
            out = []
            changed = False
            for inst in insts:
                si = getattr(inst, "sync_info", None)
                if si is not None and si.on_wait and len(si.on_wait) > max_waits:
                    waits = list(si.on_wait)
                    for w in waits[:-max_waits]:
                        nop = bass_rust.InstNoOp(name=f"wsplit-{ctr}", ins=[], outs=[])
                        ctr += 1
                        nop.engine = inst.engine
                        nop.sync_info = mybir.SyncInfo(on_wait=[w], on_update=[])
                        out.append(nop)
                    si.on_wait = waits[-max_waits:]
                    changed = True
                out.append(inst)
            if changed:
                bb.instructions = out
    return nc


_patch_tile()


def _ceil(a, b):
    return -(-a // b)


# ----------------------------------------------------------------------------
# host-side structure
# ----------------------------------------------------------------------------
def _build_structure(edge_index, n_nodes):
    src = np.asarray(edge_index[0], dtype=np.int64)
    dst = np.asarray(edge_index[1], dtype=np.int64)
    shard = n_nodes // N_CORES

    per_core = []
    for c in range(N_CORES):
        lo = c * shard
        esel = (dst >= lo) & (dst < lo + shard)
        esrc = src[esel].astype(np.int32)
        edst = (dst[esel] - lo).astype(np.int32)
        deg = np.bincount(edst, minlength=shard)
        order = np.argsort(edst, kind="stable")
        esrc_sorted = esrc[order]
        starts = np.zeros(shard + 1, dtype=np.int64)
        np.cumsum(deg, out=starts[1:])
        d_primary = np.minimum(deg, WMAX)
        w_primary = np.maximum(2, 2 * ((d_primary + 1) // 2))
        n_entries = np.maximum(1, _ceil_arr(deg, WMAX))
        per_core.append(
            dict(
                deg=deg, starts=starts, esrc=esrc_sorted, lo=lo,
                w_primary=w_primary, n_entries=n_entries,
                ov_nodes=np.where(n_entries > 1)[0],
            )
        )

    widths = list(range(2, WMAX + 1, 2))
    singles_blocks = 0
    Kb = 0
    max_entries = 1
    for st in per_core:
        max_entries = max(max_entries, int(st["n_entries"].max()))
        Kb = max(Kb, _ceil(len(st["ov_nodes"]), P))
        n16 = int(((st["w_primary"] == WMAX) & (st["n_entries"] == 1)).sum())
        singles_blocks = max(singles_blocks, _ceil(n16, P))
    nb_common = []
    for w in widths[:-1]:
        mx = 0
        for st in per_core:
            n = int(((st["w_primary"] == w) & (st["n_entries"] == 1)).sum())
            mx = max(mx, _ceil(n, P))
        nb_common.append(mx)
    nb_common.append(singles_blocks + Kb * max_entries)

    common = dict(
        class_widths=widths,
        class_nblocks=nb_common,
        ov_single=singles_blocks * P,
        ov_Kb=Kb,
        ov_max_entries=max_entries,
        E=sum(nb * P for nb in nb_common),
        TOT=sum(nb * w for nb, w in zip(nb_common, widths)),
    )

    cores = []
    for st in per_core:
        deg, starts, esrc_sorted, lo = st["deg"], st["starts"], st["esrc"], st["lo"]
        w_primary, n_entries = st["w_primary"], st["n_entries"]
        entry_node = []
        entry_eidx = []
        for w, nb in zip(widths, nb_common):
            if w < WMAX:
                nodes_w = np.where((w_primary == w) & (n_entries == 1))[0]
                lst = [(int(n), 0) for n in nodes_w]
            else:
                nodes_w = np.where((w_primary == WMAX) & (n_entries == 1))[0]
                lst = [(int(n), 0) for n in nodes_w]
                lst += [(-1, 0)] * (singles_blocks * P - len(lst))
                for e in range(max_entries):
                    sub = [
                        (int(n), e) if e < n_entries[n] else (-1, 0)
                        for n in st["ov_nodes"]
                    ]
                    sub += [(-1, 0)] * (Kb * P - len(sub))
                    lst += sub
            lst += [(-1, 0)] * (nb * P - len(lst))
            entry_node.extend(n for n, _ in lst)
            entry_eidx.extend(e for _, e in lst)
        entry_node = np.array(entry_node, dtype=np.int64)
        entry_eidx = np.array(entry_eidx, dtype=np.int64)

        TOT = common["TOT"]
        slot_src = np.full((P, TOT), -1, dtype=np.int32)
        col = 0
        epos = 0
        for w, nb in zip(widths, nb_common):
            idx = np.arange(nb * P)
            nodes = entry_node[epos + idx]
            eidxs = entry_eidx[epos + idx]
            pp = idx % P
            bb = idx // P
            valid = nodes >= 0
            nval = nodes[valid].astype(np.int64)
            for k in range(w):
                epos_k = eidxs[valid] * WMAX + k
                has = epos_k < deg[nval]
                rows = pp[valid][has]
                cols = col + bb[valid][has] * w + k
                slot_src[rows, cols] = esrc_sorted[starts[nval[has]] + epos_k[has]]
            epos += nb * P
            col += nb * w

        primary = (entry_eidx == 0) & (entry_node >= 0)
        entry_node_g = np.where(entry_node >= 0, entry_node + lo, -1).astype(np.int32)
        cores.append(dict(slot_src=slot_src, entry_node=entry_node_g, primary=primary))
    return common, cores


def _ceil_arr(a, b):
    return -(-a // b)


# ----------------------------------------------------------------------------
# bass kernel builders
# ----------------------------------------------------------------------------
def _build_launch1(nodes_pad):
    """xw_out[p, m*32+f] = (x @ [W1_rel|W1_root] + ones*[0|b1])[m*128+p, f]"""
    mchunks = nodes_pad // P
    nc = bass.Bass()
    xT = nc.dram_tensor("xT", [65, nodes_pad], mybir.dt.float32, kind="ExternalInput")
    W1 = nc.dram_tensor("W1", [65, 32], mybir.dt.float32, kind="ExternalInput")
    xw_out = nc.dram_tensor(
        "xw_out", [P, mchunks * 32], mybir.dt.bfloat16, kind="ExternalOutput"
    )
    GROUP = 16
    ngroups = _ceil(mchunks, GROUP)

    with tile.TileContext(nc) as tc:
        with (
            tc.tile_pool(name="sbuf", bufs=1) as pool,
            tc.tile_pool(name="psum", bufs=2, space="PSUM") as psum_pool,
            tc.tile_pool(name="outp", bufs=2) as outp,
        ):
            xT_sb = pool.tile([65, nodes_pad], mybir.dt.float32)
            nc.sync.dma_start(out=xT_sb[:], in_=xT[:])
            W1_sb = pool.tile([65, 32], mybir.dt.float32)
            nc.sync.dma_start(out=W1_sb[:], in_=W1[:])
            for g in range(ngroups):
                m0 = g * GROUP
                m1 = min(m0 + GROUP, mchunks)
                pt = psum_pool.tile([P, 512], mybir.dt.float32, tag="ps")
                for m in range(m0, m1):
                    nc.tensor.matmul(
                        out=pt[:, (m - m0) * 32:(m - m0 + 1) * 32],
                        lhsT=xT_sb[:, m * P:(m + 1) * P],
                        rhs=W1_sb[:],
                        start=True,
                        stop=True,
                    )
                ob = outp.tile([P, 512], mybir.dt.bfloat16, tag="ob")
                n = (m1 - m0) * 32
                nc.vector.tensor_copy(out=ob[:, :n], in_=pt[:, :n])
                nc.sync.dma_start(out=xw_out[:, m0 * 32:m1 * 32], in_=ob[:, :n])
    return nc


def _build_launch2(class_widths, class_nblocks, ov_single, ov_Kb, ov_max_entries):
    TOT = sum(nb * w for nb, w in zip(class_nblocks, class_widths))
    Eb = sum(class_nblocks)
    nc = bass.Bass()
    msgs_d = nc.dram_tensor("msgs", [P, TOT * F], mybir.dt.bfloat16, kind="ExternalInput")
    xroot_d = nc.dram_tensor("xroot", [P, Eb * F], mybir.dt.float32, kind="ExternalInput")
    cb_d = nc.dram_tensor("cb", [P, Eb * P], mybir.dt.bfloat16, kind="ExternalInput")
    p_out = nc.dram_tensor("p_out", [P, F], mybir.dt.float32, kind="ExternalOutput")

    with tile.TileContext(nc) as tc:
        with (
            tc.tile_pool(name="sbuf", bufs=1) as pool,
            tc.tile_pool(name="psum", bufs=1, space="PSUM") as psum_pool,
        ):
            cb_sb = pool.tile([P, Eb * P], mybir.dt.bfloat16)
            nc.sync.dma_start(out=cb_sb[:], in_=cb_d[:])
            xroot_sb = pool.tile([P, Eb * F], mybir.dt.float32)
            nc.sync.dma_start(out=xroot_sb[:], in_=xroot_d[:])

            agg = pool.tile([P, Eb * F], mybir.dt.float32)
            h_bf = pool.tile([P, Eb * F], mybir.dt.bfloat16)

            col = 0
            eb0 = 0
            for ci, (w, nb) in enumerate(zip(class_widths, class_nblocks)):
                if nb == 0:
                    continue
                mt = pool.tile([P, nb * w * F], mybir.dt.bfloat16, tag=f"m{ci}")
                nc.sync.dma_start(out=mt[:], in_=msgs_d[:, col * F:(col + nb * w) * F])
                agg_slice = agg[:, eb0 * F:(eb0 + nb) * F]
                mv = mt[:].rearrange("p (n q) -> p n q", q=w * F)
                if w == 2:
                    nc.vector.tensor_tensor(
                        out=agg_slice.rearrange("p (n q) -> p n q", q=F),
                        in0=mv[:, :, 0:F],
                        in1=mv[:, :, F:2 * F],
                        op=mybir.AluOpType.add,
                    )
                else:
                    half = w // 2
                    wt = pool.tile([P, nb * half * F], mybir.dt.float32, tag=f"w{ci}")
                    nc.vector.tensor_tensor(
                        out=wt[:].rearrange("p (n q) -> p n q", q=half * F),
                        in0=mv[:, :, 0:half * F],
                        in1=mv[:, :, half * F:w * F],
                        op=mybir.AluOpType.add,
                    )
                    W = half
                    while W > 1:
                        wv = wt[:].rearrange("p (n q) -> p n q", q=half * F)
                        if W % 2 == 1:
                            nc.vector.tensor_tensor(
                                out=wv[:, :, 0:F],
                                in0=wv[:, :, 0:F],
                                in1=wv[:, :, (W - 1) * F:W * F],
                                op=mybir.AluOpType.add,
                            )
                            W -= 1
                        else:
                            hw = W // 2
                            outv = (
                                agg_slice.rearrange("p (n q) -> p n q", q=F)
                                if hw == 1
                                else wv[:, :, 0:hw * F]
                            )
                            nc.vector.tensor_tensor(
                                out=outv,
                                in0=wv[:, :, 0:hw * F],
                                in1=wv[:, :, hw * F:W * F],
                                op=mybir.AluOpType.add,
                            )
                            W = hw
                col += nb * w
                eb0 += nb

            if ov_Kb > 0 and ov_max_entries > 1:
                eb_last = sum(class_nblocks[:-1])
                base = eb_last + ov_single // P
                for e in range(1, ov_max_entries):
                    b0 = base + e * ov_Kb
                    nc.vector.tensor_tensor(
                        out=agg[:, base * F:(base + ov_Kb) * F],
                        in0=agg[:, base * F:(base + ov_Kb) * F],
                        in1=agg[:, b0 * F:(b0 + ov_Kb) * F],
                        op=mybir.AluOpType.add,
                    )

            nc.vector.tensor_tensor(
                out=agg[:], in0=agg[:], in1=xroot_sb[:], op=mybir.AluOpType.add
            )
            nc.vector.tensor_scalar(
                out=h_bf[:], in0=agg[:], scalar1=0.0, scalar2=None,
                op0=mybir.AluOpType.max,
            )

            pt = psum_pool.tile([P, F], mybir.dt.float32)
            for b in range(Eb):
                nc.tensor.matmul(
                    out=pt[:],
                    lhsT=cb_sb[:, b * P:(b + 1) * P],
                    rhs=h_bf[:, b * F:(b + 1) * F],
                    start=(b == 0),
                    stop=(b == Eb - 1),
                )
            res = pool.tile([P, F], mybir.dt.float32)
            nc.vector.tensor_copy(out=res[:], in_=pt[:])
            nc.sync.dma_start(out=p_out[:], in_=res[:])
    return nc


# ----------------------------------------------------------------------------
# main entry
# ----------------------------------------------------------------------------
def kernel(**inputs) -> np.ndarray:
    x = np.asarray(inputs["x"], dtype=np.float32)
    edge_index = np.asarray(inputs["edge_index"])
    batch = np.asarray(inputs["batch"], dtype=np.int64)
    W1_rel = np.asarray(inputs["W1_rel"], dtype=np.float32)
    W1_root = np.asarray(inputs["W1_root"], dtype=np.float32)
    b1 = np.asarray(inputs["b1"], dtype=np.float32)
    W2_rel = np.asarray(inputs["W2_rel"], dtype=np.float64)
    W2_root = np.asarray(inputs["W2_root"], dtype=np.float64)
    b2 = np.asarray(inputs["b2"], dtype=np.float64)
    Wlin = np.asarray(inputs["Wlin"], dtype=np.float64)
    blin = np.asarray(inputs["blin"], dtype=np.float64)

    n_nodes, nfeat = x.shape
    shard = n_nodes // N_CORES
    nodes_pad = _ceil(shard, P) * P
    mchunks = nodes_pad // P

    common, cores = _build_structure(edge_index, n_nodes)

    # ---- launch 1 ----
    nc1 = _build_launch1(nodes_pad)
    _split_sync_waits(nc1)
    W1 = np.zeros((nfeat + 1, 32), np.float32)
    W1[:nfeat, :F] = W1_rel
    W1[:nfeat, F:] = W1_root
    W1[nfeat, F:] = b1
    in_maps1 = []
    for c in range(N_CORES):
        xs = x[c * shard:(c + 1) * shard]
        xT = np.zeros((nfeat + 1, nodes_pad), np.float32)
        xT[:nfeat, :shard] = xs.T
        xT[nfeat, :] = 1.0
        in_maps1.append({"xT": xT, "W1": W1})
    res1 = run_bass_kernel_spmd(nc1, in_maps1, list(range(N_CORES)))

    # decode per-core outputs into full-node tables
    xw_full = np.zeros((n_nodes, F), np.float32)
    xroot_full = np.zeros((n_nodes, F), np.float32)
    for c in range(N_CORES):
        dec = (
            np.asarray(res1.results[c]["xw_out"])
            .astype(np.float32)
            .reshape(P, mchunks, 32)
            .transpose(1, 0, 2)
            .reshape(nodes_pad, 32)
        )
        xw_full[c * shard:(c + 1) * shard] = dec[:shard, :F]
        xroot_full[c * shard:(c + 1) * shard] = dec[:shard, F:]

    # ---- host: expand messages + build cb tables ----
    E = common["E"]
    Eb = E // P
    gdst = batch[np.asarray(edge_index[1], dtype=np.int64)]
    src64 = np.asarray(edge_index[0], dtype=np.int64)
    Cmat = np.zeros((n_nodes, N_GRAPHS), np.float32)
    np.add.at(Cmat, (src64, gdst), 1.0)

    in_maps2 = []
    for st in cores:
        ss = st["slot_src"]
        m = np.where(ss[:, :, None] >= 0, xw_full[np.maximum(ss, 0)], 0.0)
        msgs = np.ascontiguousarray(
            m.reshape(P, -1).astype(ml_dtypes.bfloat16)
        )
        en = st["entry_node"]
        prim = st["primary"] & (en >= 0)
        xr = np.where(prim[:, None], xroot_full[np.maximum(en, 0)], 0.0).astype(
            np.float32
        )
        xroot_dev = np.ascontiguousarray(
            xr.reshape(Eb, P, F).transpose(1, 0, 2).reshape(P, Eb * F)
        )
        cb = np.zeros((E, 2 * N_GRAPHS), np.float32)
        cb[prim, :N_GRAPHS] = Cmat[en[prim]]
        cb[prim, N_GRAPHS + batch[en[prim]]] = 1.0
        cb_dev = np.ascontiguousarray(
            cb.reshape(Eb, P, 2 * N_GRAPHS)
            .transpose(1, 0, 2)
            .reshape(P, Eb * 2 * N_GRAPHS)
            .astype(ml_dtypes.bfloat16)
        )
        in_maps2.append({"msgs": msgs, "xroot": xroot_dev, "cb": cb_dev})

    # ---- launch 2 ----
    nc2 = _build_launch2(
        common["class_widths"], common["class_nblocks"], common["ov_single"],
        common["ov_Kb"], common["ov_max_entries"],
    )
    _split_sync_waits(nc2)
    res2 = run_bass_kernel_spmd(nc2, in_maps2, list(range(N_CORES)))

    # ---- host finish ----
    P_total = np.zeros((2 * N_GRAPHS, F), np.float64)
    for c in range(N_CORES):
        P_total += np.asarray(res2.results[c]["p_out"]).astype(np.float64)
    v2 = (W2_rel @ Wlin)[:, 0]
    vr = (W2_root @ Wlin)[:, 0]
    cnt = np.bincount(batch, minlength=N_GRAPHS).astype(np.float64)[:N_GRAPHS]
    cnt = np.maximum(cnt, 1.0)
    out = (P_total[:N_GRAPHS] @ v2 + P_total[N_GRAPHS:] @ vr) / cnt
    out = out + (b2 @ Wlin)[0] + blin[0]
    return out.astype(np.float32)


# revision 14
# speedup vs baseline: 1.6207x; 1.6207x over previous
"""Trainium2 Bass kernel for the 2-layer GraphConv + mean-pool network.

Self-contained: kernel(**inputs) -> np.ndarray [N_GRAPHS] float32.

Strategy (8 NeuronCores, SPMD, 2 launches):
  Launch 1: per-core node shard, compute x @ [W1_rel | W1_root] (+b1 via a
    ones-row) on the tensor engine -> per-node 16-wide projections.
    This exploits segment_sum(x[src]) @ W == segment_sum((x@W)[src]) to cut
    edge traffic 4x.
  Host: expand (x@W1_rel)[src] per edge into a degree-class-sorted,
    partition-aligned layout (pure index/permutation work), replicating the
    halo exchange. Nodes are sharded by contiguous dst ranges.
  Launch 2: per-core, stream edge messages sequentially; segmented reduction
    over each node's incident edges via log-halving strided vector adds
    (bf16 in, f32 accumulate); h = relu(agg + x@W1_root + b1); then a single
    PSUM-accumulated matmul P = [C|B]^T @ h where C[j,g] = #out-edges of node
    j landing in graph g and B = one-hot(batch). This collapses the entire
    second GraphConv layer + global mean pool into one matmul because the
    final output only needs per-graph sums (linearity of layer 2).
  Host: out[g] = (P[g]@(W2_rel@Wlin) + P[64+g]@(W2_root@Wlin)) / count_g
                 + b2@Wlin + blin   (a 64-element finish).
"""
import sys

if "/opt/trn_rl_repo" not in sys.path:
    sys.path.insert(0, "/opt/trn_rl_repo")

import numpy as np
import ml_dtypes

import concourse.bass as bass
import concourse.mybir as mybir
import concourse.tile as tile
from concourse.vector_clock import ScopedClock
from concourse.bass_utils import run_bass_kernel_spmd

N_CORES = 8
P = 128
F = 16
WMAX = 16
N_GRAPHS = 64

# ----------------------------------------------------------------------------
# toolchain workarounds
# ----------------------------------------------------------------------------
_PATCHED = False


def _patch_tile():
    """Walrus (neuronxcc) rejects >~2 sync waits on one instruction; Tile's
    final drain can carry many. Emit them as separate nops instead."""
    global _PATCHED
    if _PATCHED:
        return
    _PATCHED = True

    def patched(self, tick_clock, wait_clock):
        nop = self.nc.sync.nop(nofuse=True)
        wait_clock.add_sem_waits(nop.ins, ScopedClock({None: tick_clock.global_clock}))
        si = nop.ins.sync_info
        if si is not None and si.on_wait and len(si.on_wait) > 1:
            waits = list(si.on_wait)
            si.on_wait = waits[:1]
            for w in waits[1:]:
                nop2 = self.nc.sync.nop(nofuse=True)
                si2 = nop2.ins.sync_info
                if si2 is None:
                    nop2.ins.sync_info = mybir.SyncInfo(on_wait=[w], on_update=[])
                else:
                    si2.on_wait = [w]
        self.nc.sync.drain()
        self.nc.all_engine_barrier()
        assert self.sems is not None
        popped = self.nc._tile_sem_poison_stack.pop()
        assert popped is self._sem_poison
        self.nc.clear_and_free_semaphores(list(self.sems.allocated().values()))
        self.nc.all_engine_barrier()

    tile.TileContext._drain_and_barrier = patched


def _split_sync_waits(nc, max_waits=1):
    """Move excess per-instruction sync waits onto injected NoOps."""
    import bass_rust

    ctr = 0
    for fn in nc.m.functions:
        for bb in fn.blocks:
            insts = list(bb.instructions)
            out = []
            changed = False
            for inst in insts:
                si = getattr(inst, "sync_info", None)
                if si is not None and si.on_wait and len(si.on_wait) > max_waits:
                    waits = list(si.on_wait)
                    for w in waits[:-max_waits]:
                        nop = bass_rust.InstNoOp(name=f"wsplit-{ctr}", ins=[], outs=[])
                        ctr += 1
                        nop.engine = inst.engine
                        nop.sync_info = mybir.SyncInfo(on_wait=[w], on_update=[])
                        out.append(nop)
                    si.on_wait = waits[-max_waits:]
                    changed = True
                out.append(inst)
            if changed:
                bb.instructions = out
    return nc


_patch_tile()


def _ceil(a, b):
    return -(-a // b)


# ----------------------------------------------------------------------------
# host-side structure
# ----------------------------------------------------------------------------
def _build_structure(edge_index, n_nodes):
    src = np.asarray(edge_index[0], dtype=np.int64)
    dst = np.asarray(edge_index[1], dtype=np.int64)
    shard = n_nodes // N_CORES

    per_core = []
    for c in range(N_CORES):
        lo = c * shard
        esel = (dst >= lo) & (dst < lo + shard)
        esrc = src[esel].astype(np.int32)
        edst = (dst[esel] - lo).astype(np.int32)
        deg = np.bincount(edst, minlength=shard)
        order = np.argsort(edst, kind="stable")
        esrc_sorted = esrc[order]
        starts = np.zeros(shard + 1, dtype=np.int64)
        np.cumsum(deg, out=starts[1:])
        d_primary = np.minimum(deg, WMAX)
        w_primary = np.maximum(2, 2 * ((d_primary + 1) // 2))
        n_entries = np.maximum(1, _ceil_arr(deg, WMAX))
        per_core.append(
            dict(
                deg=deg, starts=starts, esrc=esrc_sorted, lo=lo,
                w_primary=w_primary, n_entries=n_entries,
                ov_nodes=np.where(n_entries > 1)[0],
            )
        )

    widths = list(range(2, WMAX + 1, 2))
    singles_blocks = 0
    Kb = 0
    max_entries = 1
    for st in per_core:
        max_entries = max(max_entries, int(st["n_entries"].max()))
        Kb = max(Kb, _ceil(len(st["ov_nodes"]), P))
        n16 = int(((st["w_primary"] == WMAX) & (st["n_entries"] == 1)).sum())
        singles_blocks = max(singles_blocks, _ceil(n16, P))
    nb_common = []
    for w in widths[:-1]:
        mx = 0
        for st in per_core:
            n = int(((st["w_primary"] == w) & (st["n_entries"] == 1)).sum())
            mx = max(mx, _ceil(n, P))
        nb_common.append(mx)
    nb_common.append(singles_blocks + Kb * max_entries)

    common = dict(
        class_widths=widths,
        class_nblocks=nb_common,
        ov_single=singles_blocks * P,
        ov_Kb=Kb,
        ov_max_entries=max_entries,
        E=sum(nb * P for nb in nb_common),
        TOT=sum(nb * w for nb, w in zip(nb_common, widths)),
    )

    cores = []
    for st in per_core:
        deg, starts, esrc_sorted, lo = st["deg"], st["starts"], st["esrc"], st["lo"]
        w_primary, n_entries = st["w_primary"], st["n_entries"]
        entry_node = []
        entry_eidx = []
        for w, nb in zip(widths, nb_common):
            if w < WMAX:
                nodes_w = np.where((w_primary == w) & (n_entries == 1))[0]
                lst = [(int(n), 0) for n in nodes_w]
            else:
                nodes_w = np.where((w_primary == WMAX) & (n_entries == 1))[0]
                lst = [(int(n), 0) for n in nodes_w]
                lst += [(-1, 0)] * (singles_blocks * P - len(lst))
                for e in range(max_entries):
                    sub = [
                        (int(n), e) if e < n_entries[n] else (-1, 0)
                        for n in st["ov_nodes"]
                    ]
                    sub += [(-1, 0)] * (Kb * P - len(sub))
                    lst += sub
            lst += [(-1, 0)] * (nb * P - len(lst))
            entry_node.extend(n for n, _ in lst)
            entry_eidx.extend(e for _, e in lst)
        entry_node = np.array(entry_node, dtype=np.int64)
        entry_eidx = np.array(entry_eidx, dtype=np.int64)

        TOT = common["TOT"]
        slot_src = np.full((P, TOT), -1, dtype=np.int32)
        col = 0
        epos = 0
        for w, nb in zip(widths, nb_common):
            idx = np.arange(nb * P)
            nodes = entry_node[epos + idx]
            eidxs = entry_eidx[epos + idx]
            pp = idx % P
            bb = idx // P
            valid = nodes >= 0
            nval = nodes[valid].astype(np.int64)
            for k in range(w):
                epos_k = eidxs[valid] * WMAX + k
                has = epos_k < deg[nval]
                rows = pp[valid][has]
                cols = col + bb[valid][has] * w + k
                slot_src[rows, cols] = esrc_sorted[starts[nval[has]] + epos_k[has]]
            epos += nb * P
            col += nb * w

        primary = (entry_eidx == 0) & (entry_node >= 0)
        entry_node_g = np.where(entry_node >= 0, entry_node + lo, -1).astype(np.int32)
        cores.append(dict(slot_src=slot_src, entry_node=entry_node_g, primary=primary))
    return common, cores


def _ceil_arr(a, b):
    return -(-a // b)


# ----------------------------------------------------------------------------
# bass kernel builders
# ----------------------------------------------------------------------------
def _build_launch1(nodes_pad, nfeat=64):
    """xw_out[p, m*32+f] = (x @ [W1_rel|W1_root])[m*128+p, f]

    x is bf16; full f32 weight precision is recovered by splitting
    W = W_hi + W_lo (both bf16) and accumulating two matmuls in PSUM.
    xT is loaded in 3 column chunks (DMA count dominates fixed cost);
    the output is staged into one SBUF tile and written with one DMA."""
    mchunks = nodes_pad // P
    nc = bass.Bass()
    xT = nc.dram_tensor("xT", [nfeat, nodes_pad], mybir.dt.bfloat16, kind="ExternalInput")
    W1 = nc.dram_tensor("W1", [nfeat, 64], mybir.dt.bfloat16, kind="ExternalInput")
    xw_out = nc.dram_tensor(
        "xw_out", [P, mchunks * 32], mybir.dt.bfloat16, kind="ExternalOutput"
    )
    NDMA = 3
    per = _ceil(mchunks, NDMA)
    GROUP = 16  # chunks per psum bank (16*32 = 512 f32)

    with tile.TileContext(nc) as tc:
        with (
            tc.tile_pool(name="sbuf", bufs=1) as pool,
            tc.tile_pool(name="xtp", bufs=3) as xtp,
            tc.tile_pool(name="psum", bufs=4, space="PSUM") as psum_pool,
        ):
            W1_sb = pool.tile([nfeat, 64], mybir.dt.bfloat16)
            nc.sync.dma_start(out=W1_sb[:], in_=W1[:])
            out_sb = pool.tile([P, mchunks * 32], mybir.dt.bfloat16)
            for d in range(NDMA):
                c0 = d * per
                c1 = min(c0 + per, mchunks)
                xt_g = xtp.tile([nfeat, (c1 - c0) * P], mybir.dt.bfloat16, tag="xt")
                nc.sync.dma_start(out=xt_g[:], in_=xT[:, c0 * P:c1 * P])
                for g0 in range(c0, c1, GROUP):
                    g1 = min(g0 + GROUP, c1)
                    pt = psum_pool.tile([P, 512], mybir.dt.float32, tag="ps")
                    for m in range(g0, g1):
                        sl = pt[:, (m - g0) * 32:(m - g0 + 1) * 32]
                        lhs = xt_g[:, (m - c0) * P:(m - c0 + 1) * P]
                        nc.tensor.matmul(out=sl, lhsT=lhs, rhs=W1_sb[:, 0:32],
                                         start=True, stop=False)
                        nc.tensor.matmul(out=sl, lhsT=lhs, rhs=W1_sb[:, 32:64],
                                         start=False, stop=True)
                    n = (g1 - g0) * 32
                    # NOTE: ACT (scalar) reading PSUM crashes the device on
                    # this toolchain -- keep this copy on DVE.
                    nc.vector.tensor_copy(
                        out=out_sb[:, g0 * 32:g0 * 32 + n], in_=pt[:, :n]
                    )
            nc.sync.dma_start(out=xw_out[:], in_=out_sb[:])
    return nc


def _build_launch2(class_widths, class_nblocks, ov_single, ov_Kb, ov_max_entries):
    """Per-core launch 2. One packed uint8 "blob" input; per class one DMA
    covering [msgs bf16 | cb f8 | xroot bf16] segments (bitcast views)."""
    Eb = sum(class_nblocks)
    # per-class byte layout (per partition row)
    seg = []  # (off_msgs, off_cb, off_xr, bytes_c)
    off = 0
    for w, nb in zip(class_widths, class_nblocks):
        bm = nb * w * F * 2
        bc = nb * P
        bx = nb * F * 2
        seg.append((off, off + bm, off + bm + bc, bm + bc + bx))
        off += bm + bc + bx
    total_bytes = off

    nc = bass.Bass()
    blob_d = nc.dram_tensor("blob", [P, total_bytes], mybir.dt.uint8, kind="ExternalInput")
    p_out = nc.dram_tensor("p_out", [P, F], mybir.dt.float32, kind="ExternalOutput")

    ncls = len(class_widths)
    with tile.TileContext(nc) as tc:
        with (
            tc.tile_pool(name="sbuf", bufs=1) as pool,
            tc.tile_pool(name="psum", bufs=1, space="PSUM") as psum_pool,
        ):
            pt = psum_pool.tile([P, F], mybir.dt.float32)
            eb0 = 0
            bglobal = 0
            for ci, (w, nb) in enumerate(zip(class_widths, class_nblocks)):
                if nb == 0:
                    continue
                last_class = ci == ncls - 1
                o_m, o_c, o_x, nbytes = seg[ci]
                bt = pool.tile([P, nbytes], mybir.dt.uint8, tag=f"b{ci}")
                nc.sync.dma_start(out=bt[:], in_=blob_d[:, o_m:o_m + nbytes])
                mv = bt[:, 0:nb * w * F * 2].bitcast(mybir.dt.bfloat16).rearrange(
                    "p (n q) -> p n q", q=w * F
                )
                cb_v = bt[:, o_c - o_m:o_c - o_m + nb * P].bitcast(mybir.dt.float8e4)
                xr_v = bt[:, o_x - o_m:o_x - o_m + nb * F * 2].bitcast(mybir.dt.bfloat16)

                agg_c = pool.tile([P, nb * F], mybir.dt.bfloat16, tag=f"a{ci}")
                agg_v = agg_c[:].rearrange("p (n q) -> p n q", q=F)
                if w == 2:
                    nc.vector.tensor_tensor(
                        out=agg_v, in0=mv[:, :, 0:F], in1=mv[:, :, F:2 * F],
                        op=mybir.AluOpType.add,
                    )
                else:
                    half = w // 2
                    wt = pool.tile([P, nb * half * F], mybir.dt.bfloat16, tag=f"w{ci}")
                    nc.vector.tensor_tensor(
                        out=wt[:].rearrange("p (n q) -> p n q", q=half * F),
                        in0=mv[:, :, 0:half * F],
                        in1=mv[:, :, half * F:w * F],
                        op=mybir.AluOpType.add,
                    )
                    W = half
                    while W > 1:
                        wv = wt[:].rearrange("p (n q) -> p n q", q=half * F)
                        if W % 2 == 1:
                            nc.vector.tensor_tensor(
                                out=wv[:, :, 0:F],
                                in0=wv[:, :, 0:F],
                                in1=wv[:, :, (W - 1) * F:W * F],
                                op=mybir.AluOpType.add,
                            )
                            W -= 1
                        else:
                            hw = W // 2
                            outv = agg_v if hw == 1 else wv[:, :, 0:hw * F]
                            nc.vector.tensor_tensor(
                                out=outv,
                                in0=wv[:, :, 0:hw * F],
                                in1=wv[:, :, hw * F:W * F],
                                op=mybir.AluOpType.add,
                            )
                            W = hw

                if last_class and ov_Kb > 0 and ov_max_entries > 1:
                    base = ov_single // P
                    for e in range(1, ov_max_entries):
                        b0 = base + e * ov_Kb
                        nc.vector.tensor_tensor(
                            out=agg_c[:, base * F:(base + ov_Kb) * F],
                            in0=agg_c[:, base * F:(base + ov_Kb) * F],
                            in1=agg_c[:, b0 * F:(b0 + ov_Kb) * F],
                            op=mybir.AluOpType.add,
                        )

                # h = relu(agg + xroot) -> bf16 (relu on ACT to offload DVE)
                nc.vector.tensor_tensor(
                    out=agg_c[:], in0=agg_c[:], in1=xr_v, op=mybir.AluOpType.add
                )
                h_c = pool.tile([P, nb * F], mybir.dt.bfloat16, tag=f"h{ci}")
                nc.scalar.activation(
                    out=h_c[:], in_=agg_c[:], func=mybir.ActivationFunctionType.Relu
                )
                for b in range(nb):
                    nc.tensor.matmul(
                        out=pt[:],
                        lhsT=cb_v[:, b * P:(b + 1) * P],
                        rhs=h_c[:, b * F:(b + 1) * F],
                        start=(bglobal == 0),
                        stop=(bglobal == Eb - 1),
                    )
                    bglobal += 1
                eb0 += nb

            res = pool.tile([P, F], mybir.dt.float32)
            nc.vector.tensor_copy(out=res[:], in_=pt[:])
            nc.sync.dma_start(out=p_out[:], in_=res[:])
    return nc


# ----------------------------------------------------------------------------
# main entry
# ----------------------------------------------------------------------------
def kernel(**inputs) -> np.ndarray:
    x = np.asarray(inputs["x"], dtype=np.float32)
    edge_index = np.asarray(inputs["edge_index"])
    batch = np.asarray(inputs["batch"], dtype=np.int64)
    W1_rel = np.asarray(inputs["W1_rel"], dtype=np.float32)
    W1_root = np.asarray(inputs["W1_root"], dtype=np.float32)
    b1 = np.asarray(inputs["b1"], dtype=np.float32)
    W2_rel = np.asarray(inputs["W2_rel"], dtype=np.float64)
    W2_root = np.asarray(inputs["W2_root"], dtype=np.float64)
    b2 = np.asarray(inputs["b2"], dtype=np.float64)
    Wlin = np.asarray(inputs["Wlin"], dtype=np.float64)
    blin = np.asarray(inputs["blin"], dtype=np.float64)

    n_nodes, nfeat = x.shape
    shard = n_nodes // N_CORES
    nodes_pad = _ceil(shard, P) * P
    mchunks = nodes_pad // P

    common, cores = _build_structure(edge_index, n_nodes)

    # ---- launch 1 ----
    nc1 = _build_launch1(nodes_pad, nfeat)
    _split_sync_waits(nc1)
    # W = W_hi + W_lo (both bf16) recovers f32 weight precision via two
    # PSUM-accumulated matmuls.
    Wcat = np.concatenate([W1_rel, W1_root], axis=1)  # [64, 32]
    W_hi = Wcat.astype(ml_dtypes.bfloat16)
    W_lo = (Wcat - W_hi.astype(np.float32)).astype(ml_dtypes.bfloat16)
    W1 = np.concatenate([W_hi, W_lo], axis=1)  # [64, 64] bf16
    in_maps1 = []
    for c in range(N_CORES):
        xs = x[c * shard:(c + 1) * shard]
        xT = np.zeros((nfeat, nodes_pad), ml_dtypes.bfloat16)
        xT[:, :shard] = xs.T.astype(ml_dtypes.bfloat16)
        in_maps1.append({"xT": xT, "W1": W1})
    res1 = run_bass_kernel_spmd(nc1, in_maps1, list(range(N_CORES)))

    # decode per-core outputs into full-node tables
    xw_full = np.zeros((n_nodes, F), np.float32)
    xroot_full = np.zeros((n_nodes, F), np.float32)
    for c in range(N_CORES):
        dec = (
            np.asarray(res1.results[c]["xw_out"])
            .astype(np.float32)
            .reshape(P, mchunks, 32)
            .transpose(1, 0, 2)
            .reshape(nodes_pad, 32)
        )
        xw_full[c * shard:(c + 1) * shard] = dec[:shard, :F]
        xroot_full[c * shard:(c + 1) * shard] = dec[:shard, F:]

    # ---- host: expand messages + build cb tables ----
    E = common["E"]
    Eb = E // P
    gdst = batch[np.asarray(edge_index[1], dtype=np.int64)]
    src64 = np.asarray(edge_index[0], dtype=np.int64)
    Cmat = np.zeros((n_nodes, N_GRAPHS), np.float32)
    np.add.at(Cmat, (src64, gdst), 1.0)

    in_maps2 = []
    cw, cn = common["class_widths"], common["class_nblocks"]
    for st in cores:
        ss = st["slot_src"]
        m = np.where(ss[:, :, None] >= 0, xw_full[np.maximum(ss, 0)], 0.0)
        msgs = np.ascontiguousarray(
            m.reshape(P, -1).astype(ml_dtypes.bfloat16)
        )
        en = st["entry_node"]
        prim = st["primary"] & (en >= 0)
        # xroot' = x@W1_root + b1 (bias folded here on host)
        xr = np.where(
            prim[:, None], xroot_full[np.maximum(en, 0)] + b1, 0.0
        ).astype(np.float32)
        xroot_dev = np.ascontiguousarray(
            xr.reshape(Eb, P, F).transpose(1, 0, 2).reshape(P, Eb * F)
        ).astype(ml_dtypes.bfloat16)
        cb = np.zeros((E, 2 * N_GRAPHS), np.float32)
        cb[prim, :N_GRAPHS] = Cmat[en[prim]]
        cb[prim, N_GRAPHS + batch[en[prim]]] = 1.0
        cb_dev = np.ascontiguousarray(
            cb.reshape(Eb, P, 2 * N_GRAPHS)
            .transpose(1, 0, 2)
            .reshape(P, Eb * 2 * N_GRAPHS)
            .astype(ml_dtypes.float8_e4m3fn)
        )
        # pack per-class blob: [msgs_c | cb_c | xroot_c] bytes per class
        parts = []
        col = 0
        eb0 = 0
        for w, nb in zip(cw, cn):
            parts.append(msgs[:, (col * F):(col + nb * w) * F].view(np.uint8))
            parts.append(cb_dev[:, eb0 * P:(eb0 + nb) * P].view(np.uint8))
            parts.append(xroot_dev[:, eb0 * F:(eb0 + nb) * F].view(np.uint8))
            col += nb * w
            eb0 += nb
        blob = np.ascontiguousarray(np.concatenate(parts, axis=1))
        in_maps2.append({"blob": blob})

    # ---- launch 2 ----
    nc2 = _build_launch2(
        common["class_widths"], common["class_nblocks"], common["ov_single"],
        common["ov_Kb"], common["ov_max_entries"],
    )
    _split_sync_waits(nc2)
    res2 = run_bass_kernel_spmd(nc2, in_maps2, list(range(N_CORES)))

    # ---- host finish ----
    P_total = np.zeros((2 * N_GRAPHS, F), np.float64)
    for c in range(N_CORES):
        P_total += np.asarray(res2.results[c]["p_out"]).astype(np.float64)
    v2 = (W2_rel @ Wlin)[:, 0]
    vr = (W2_root @ Wlin)[:, 0]
    cnt = np.bincount(batch, minlength=N_GRAPHS).astype(np.float64)[:N_GRAPHS]
    cnt = np.maximum(cnt, 1.0)
    out = (P_total[:N_GRAPHS] @ v2 + P_total[N_GRAPHS:] @ vr) / cnt
    out = out + (b2 @ Wlin)[0] + blin[0]
    return out.astype(np.float32)


# revision 20
# speedup vs baseline: 1.7375x; 1.0721x over previous
"""Trainium2 Bass kernel for the 2-layer GraphConv + mean-pool network.

Self-contained: kernel(**inputs) -> np.ndarray [N_GRAPHS] float32.

Strategy (8 NeuronCores, SPMD, 2 launches):
  Launch 1: per-core node shard, compute x @ [W1_rel | W1_root] on the
    tensor engine (bf16 x; f32 weight precision recovered via a
    W = W_hi + W_lo bf16 residual split, two PSUM-accumulated matmuls).
    This exploits segment_sum(x[src]) @ W == segment_sum((x@W)[src]) to cut
    edge traffic 4x. b1 is folded into the xroot staging on the host.
  Host: expand (x@W1_rel)[src] per edge into a degree-class-sorted,
    partition-aligned layout (pure index/permutation work), replicating the
    halo exchange. Nodes are sharded by contiguous dst ranges.
  Launch 2: per-core, stream edge messages sequentially; segmented reduction
    over each node's incident edges via log-halving strided vector adds
    (bf16 in, f32 accumulate); h = relu(agg + x@W1_root + b1); then a single
    PSUM-accumulated matmul P = [C|B]^T @ h where C[j,g] = #out-edges of node
    j landing in graph g and B = one-hot(batch). This collapses the entire
    second GraphConv layer + global mean pool into one matmul because the
    final output only needs per-graph sums (linearity of layer 2).
  Host: out[g] = (P[g]@(W2_rel@Wlin) + P[64+g]@(W2_root@Wlin)) / count_g
                 + b2@Wlin + blin   (a 64-element finish).
"""
import sys

if "/opt/trn_rl_repo" not in sys.path:
    sys.path.insert(0, "/opt/trn_rl_repo")

import numpy as np
import ml_dtypes

import concourse.bass as bass
import concourse.mybir as mybir
import concourse.tile as tile
from concourse.vector_clock import ScopedClock
from concourse.bass_utils import run_bass_kernel_spmd

N_CORES = 8
P = 128
F = 16
WMAX = 16
N_GRAPHS = 64

# ----------------------------------------------------------------------------
# toolchain workarounds
# ----------------------------------------------------------------------------
_PATCHED = False


def _patch_tile():
    """Walrus (neuronxcc) rejects >~2 sync waits on one instruction; Tile's
    final drain can carry many. Emit them as separate nops instead."""
    global _PATCHED
    if _PATCHED:
        return
    _PATCHED = True

    def patched(self, tick_clock, wait_clock):
        nop = self.nc.sync.nop(nofuse=True)
        wait_clock.add_sem_waits(nop.ins, ScopedClock({None: tick_clock.global_clock}))
        si = nop.ins.sync_info
        if si is not None and si.on_wait and len(si.on_wait) > 1:
            waits = list(si.on_wait)
            si.on_wait = waits[:1]
            for w in waits[1:]:
                nop2 = self.nc.sync.nop(nofuse=True)
                si2 = nop2.ins.sync_info
                if si2 is None:
                    nop2.ins.sync_info = mybir.SyncInfo(on_wait=[w], on_update=[])
                else:
                    si2.on_wait = [w]
        self.nc.sync.drain()
        self.nc.all_engine_barrier()
        assert self.sems is not None
        popped = self.nc._tile_sem_poison_stack.pop()
        assert popped is self._sem_poison
        self.nc.clear_and_free_semaphores(list(self.sems.allocated().values()))
        self.nc.all_engine_barrier()

    tile.TileContext._drain_and_barrier = patched


def _split_sync_waits(nc, max_waits=1):
    """Move excess per-instruction sync waits onto injected NoOps."""
    import bass_rust

    ctr = 0
    for fn in nc.m.functions:
        for bb in fn.blocks:
            insts = list(bb.instructions)
            out = []
            changed = False
            for inst in insts:
                si = getattr(inst, "sync_info", None)
                if si is not None and si.on_wait and len(si.on_wait) > max_waits:
                    waits = list(si.on_wait)
                    for w in waits[:-max_waits]:
                        nop = bass_rust.InstNoOp(name=f"wsplit-{ctr}", ins=[], outs=[])
                        ctr += 1
                        nop.engine = inst.engine
                        nop.sync_info = mybir.SyncInfo(on_wait=[w], on_update=[])
                        out.append(nop)
                    si.on_wait = waits[-max_waits:]
                    changed = True
                out.append(inst)
            if changed:
                bb.instructions = out
    return nc


_patch_tile()


def _ceil(a, b):
    return -(-a // b)


# ----------------------------------------------------------------------------
# host-side structure
# ----------------------------------------------------------------------------
def _build_structure(edge_index, n_nodes):
    src = np.asarray(edge_index[0], dtype=np.int64)
    dst = np.asarray(edge_index[1], dtype=np.int64)
    shard = n_nodes // N_CORES

    per_core = []
    for c in range(N_CORES):
        lo = c * shard
        esel = (dst >= lo) & (dst < lo + shard)
        esrc = src[esel].astype(np.int32)
        edst = (dst[esel] - lo).astype(np.int32)
        deg = np.bincount(edst, minlength=shard)
        order = np.argsort(edst, kind="stable")
        esrc_sorted = esrc[order]
        starts = np.zeros(shard + 1, dtype=np.int64)
        np.cumsum(deg, out=starts[1:])
        d_primary = np.minimum(deg, WMAX)
        w_primary = np.maximum(2, 2 * ((d_primary + 1) // 2))
        n_entries = np.maximum(1, _ceil_arr(deg, WMAX))
        per_core.append(
            dict(
                deg=deg, starts=starts, esrc=esrc_sorted, lo=lo,
                w_primary=w_primary, n_entries=n_entries,
                ov_nodes=np.where(n_entries > 1)[0],
            )
        )

    widths = list(range(2, WMAX + 1, 2))
    singles_blocks = 0
    Kb = 0
    max_entries = 1
    for st in per_core:
        max_entries = max(max_entries, int(st["n_entries"].max()))
        Kb = max(Kb, _ceil(len(st["ov_nodes"]), P))
        n16 = int(((st["w_primary"] == WMAX) & (st["n_entries"] == 1)).sum())
        singles_blocks = max(singles_blocks, _ceil(n16, P))
    nb_common = []
    for w in widths[:-1]:
        mx = 0
        for st in per_core:
            n = int(((st["w_primary"] == w) & (st["n_entries"] == 1)).sum())
            mx = max(mx, _ceil(n, P))
        nb_common.append(mx)
    nb_common.append(singles_blocks + Kb * max_entries)

    common = dict(
        class_widths=widths,
        class_nblocks=nb_common,
        ov_single=singles_blocks * P,
        ov_Kb=Kb,
        ov_max_entries=max_entries,
        E=sum(nb * P for nb in nb_common),
        TOT=sum(nb * w for nb, w in zip(nb_common, widths)),
    )

    cores = []
    for st in per_core:
        deg, starts, esrc_sorted, lo = st["deg"], st["starts"], st["esrc"], st["lo"]
        w_primary, n_entries = st["w_primary"], st["n_entries"]
        entry_node = []
        entry_eidx = []
        for w, nb in zip(widths, nb_common):
            if w < WMAX:
                nodes_w = np.where((w_primary == w) & (n_entries == 1))[0]
                lst = [(int(n), 0) for n in nodes_w]
            else:
                nodes_w = np.where((w_primary == WMAX) & (n_entries == 1))[0]
                lst = [(int(n), 0) for n in nodes_w]
                lst += [(-1, 0)] * (singles_blocks * P - len(lst))
                for e in range(max_entries):
                    sub = [
                        (int(n), e) if e < n_entries[n] else (-1, 0)
                        for n in st["ov_nodes"]
                    ]
                    sub += [(-1, 0)] * (Kb * P - len(sub))
                    lst += sub
            lst += [(-1, 0)] * (nb * P - len(lst))
            entry_node.extend(n for n, _ in lst)
            entry_eidx.extend(e for _, e in lst)
        entry_node = np.array(entry_node, dtype=np.int64)
        entry_eidx = np.array(entry_eidx, dtype=np.int64)

        TOT = common["TOT"]
        slot_src = np.full((P, TOT), -1, dtype=np.int32)
        col = 0
        epos = 0
        for w, nb in zip(widths, nb_common):
            idx = np.arange(nb * P)
            nodes = entry_node[epos + idx]
            eidxs = entry_eidx[epos + idx]
            pp = idx % P
            bb = idx // P
            valid = nodes >= 0
            nval = nodes[valid].astype(np.int64)
            for k in range(w):
                epos_k = eidxs[valid] * WMAX + k
                has = epos_k < deg[nval]
                rows = pp[valid][has]
                cols = col + bb[valid][has] * w + k
                slot_src[rows, cols] = esrc_sorted[starts[nval[has]] + epos_k[has]]
            epos += nb * P
            col += nb * w

        primary = (entry_eidx == 0) & (entry_node >= 0)
        entry_node_g = np.where(entry_node >= 0, entry_node + lo, -1).astype(np.int32)
        cores.append(dict(slot_src=slot_src, entry_node=entry_node_g, primary=primary))
    return common, cores


def _ceil_arr(a, b):
    return -(-a // b)


# ----------------------------------------------------------------------------
# bass kernel builders
# ----------------------------------------------------------------------------
def _build_launch1(nodes_pad, nfeat=64):
    """xw_out[p, m*32+f] = (x @ [W1_rel|W1_root])[m*128+p, f]

    x is bf16; full f32 weight precision is recovered by splitting
    W = W_hi + W_lo (both bf16) and accumulating two matmuls in PSUM.
    xT is loaded in 3 column chunks (DMA count dominates fixed cost);
    the output is staged into one SBUF tile and written with one DMA."""
    mchunks = nodes_pad // P
    nc = bass.Bass()
    xT = nc.dram_tensor("xT", [nfeat, nodes_pad], mybir.dt.bfloat16, kind="ExternalInput")
    W1 = nc.dram_tensor("W1", [nfeat, 64], mybir.dt.bfloat16, kind="ExternalInput")
    xw_out = nc.dram_tensor(
        "xw_out", [P, mchunks * 32], mybir.dt.bfloat16, kind="ExternalOutput"
    )
    NDMA = 3
    per = _ceil(mchunks, NDMA)
    GROUP = 16  # chunks per psum bank (16*32 = 512 f32)

    with tile.TileContext(nc) as tc:
        with (
            tc.tile_pool(name="sbuf", bufs=1) as pool,
            tc.tile_pool(name="xtp", bufs=3) as xtp,
            tc.tile_pool(name="psum", bufs=4, space="PSUM") as psum_pool,
        ):
            W1_sb = pool.tile([nfeat, 64], mybir.dt.bfloat16)
            nc.sync.dma_start(out=W1_sb[:], in_=W1[:])
            out_sb = pool.tile([P, mchunks * 32], mybir.dt.bfloat16)
            for d in range(NDMA):
                c0 = d * per
                c1 = min(c0 + per, mchunks)
                xt_g = xtp.tile([nfeat, (c1 - c0) * P], mybir.dt.bfloat16, tag="xt")
                nc.sync.dma_start(out=xt_g[:], in_=xT[:, c0 * P:c1 * P])
                for g0 in range(c0, c1, GROUP):
                    g1 = min(g0 + GROUP, c1)
                    pt = psum_pool.tile([P, 512], mybir.dt.float32, tag="ps")
                    for m in range(g0, g1):
                        sl = pt[:, (m - g0) * 32:(m - g0 + 1) * 32]
                        lhs = xt_g[:, (m - c0) * P:(m - c0 + 1) * P]
                        nc.tensor.matmul(out=sl, lhsT=lhs, rhs=W1_sb[:, 0:32],
                                         start=True, stop=False)
                        nc.tensor.matmul(out=sl, lhsT=lhs, rhs=W1_sb[:, 32:64],
                                         start=False, stop=True)
                    n = (g1 - g0) * 32
                    # NOTE: ACT (scalar) reading PSUM crashes the device on
                    # this toolchain -- keep this copy on DVE.
                    nc.vector.tensor_copy(
                        out=out_sb[:, g0 * 32:g0 * 32 + n], in_=pt[:, :n]
                    )
            # two output DMAs: first half leaves while the tail computes
            half_m = (mchunks // 2) * 32
            nc.sync.dma_start(out=xw_out[:, :half_m], in_=out_sb[:, :half_m])
            nc.sync.dma_start(out=xw_out[:, half_m:], in_=out_sb[:, half_m:])
    return nc


def _class_plan(class_widths, class_nblocks, max_nb=10):
    """Emission plan for launch 2: split big classes into block-chunks and
    order them largest-first so the pipeline tail lands on small chunks.
    Each entry: (col0, eb0, w, nb) with offsets in the ORIGINAL layout.
    The last original class (the WMAX/overflow class) is never split and is
    flagged so the overflow combine runs inside it."""
    ncls = len(class_widths)
    chunks = []
    col = 0
    eb0 = 0
    for ci, (w, nb) in enumerate(zip(class_widths, class_nblocks)):
        if nb == 0:
            col += nb * w
            eb0 += nb
            continue
        is_ov = ci == ncls - 1
        if is_ov or nb <= max_nb:
            parts = [nb]
        else:
            k = _ceil(nb, max_nb)
            base = nb // k
            rem = nb % k
            parts = [base + (1 if i < rem else 0) for i in range(k)]
        o_col, o_eb = col, eb0
        for pnb in parts:
            chunks.append((o_col, o_eb, w, pnb, is_ov))
            o_col += pnb * w
            o_eb += pnb
        col += nb * w
        eb0 += nb
    # largest data first; overflow chunk pinned to the end of the big ones?
    # simple: sort by descending message bytes, ov chunk sorts with them.
    chunks.sort(key=lambda c: -(c[3] * c[2]))
    return chunks


def _build_launch2(class_widths, class_nblocks, ov_single, ov_Kb, ov_max_entries,
                   max_nb=10):
    """Per-core launch 2. One packed uint8 "blob" input; one DMA per plan
    chunk covering [msgs bf16 | cb f8 | xroot bf16] segments (bitcast views).
    Chunks are emitted largest-first (see _class_plan)."""
    Eb = sum(class_nblocks)
    plan = _class_plan(class_widths, class_nblocks, max_nb)
    # blob layout: per plan chunk, [msgs | cb | xroot] bytes, in plan order
    seg = []
    off = 0
    for (col0, eb0, w, nb, is_ov) in plan:
        bm = nb * w * F * 2
        bc = nb * P
        bx = nb * F * 2
        seg.append((off, off + bm, off + bm + bc, bm + bc + bx))
        off += bm + bc + bx
    total_bytes = off

    nc = bass.Bass()
    blob_d = nc.dram_tensor("blob", [P, total_bytes], mybir.dt.uint8, kind="ExternalInput")
    p_out = nc.dram_tensor("p_out", [P, F], mybir.dt.float32, kind="ExternalOutput")

    with tile.TileContext(nc) as tc:
        with (
            tc.tile_pool(name="sbuf", bufs=1) as pool,
            tc.tile_pool(name="psum", bufs=1, space="PSUM") as psum_pool,
        ):
            pt = psum_pool.tile([P, F], mybir.dt.float32)
            bglobal = 0
            for ki, ((col0, eb0, w, nb, is_ov), (o_m, o_c, o_x, nbytes)) in enumerate(
                zip(plan, seg)
            ):
                bt = pool.tile([P, nbytes], mybir.dt.uint8, tag=f"b{ki}")
                nc.sync.dma_start(out=bt[:], in_=blob_d[:, o_m:o_m + nbytes])
                mv = bt[:, 0:nb * w * F * 2].bitcast(mybir.dt.bfloat16).rearrange(
                    "p (n q) -> p n q", q=w * F
                )
                cb_v = bt[:, o_c - o_m:o_c - o_m + nb * P].bitcast(mybir.dt.float8e4)
                xr_v = bt[:, o_x - o_m:o_x - o_m + nb * F * 2].bitcast(mybir.dt.bfloat16)

                agg_c = pool.tile([P, nb * F], mybir.dt.bfloat16, tag=f"a{ki}")
                agg_v = agg_c[:].rearrange("p (n q) -> p n q", q=F)
                if w == 2:
                    nc.vector.tensor_tensor(
                        out=agg_v, in0=mv[:, :, 0:F], in1=mv[:, :, F:2 * F],
                        op=mybir.AluOpType.add,
                    )
                else:
                    half = w // 2
                    wt = pool.tile([P, nb * half * F], mybir.dt.bfloat16, tag=f"w{ki}")
                    nc.vector.tensor_tensor(
                        out=wt[:].rearrange("p (n q) -> p n q", q=half * F),
                        in0=mv[:, :, 0:half * F],
                        in1=mv[:, :, half * F:w * F],
                        op=mybir.AluOpType.add,
                    )
                    W = half
                    while W > 1:
                        wv = wt[:].rearrange("p (n q) -> p n q", q=half * F)
                        if W % 2 == 1:
                            nc.vector.tensor_tensor(
                                out=wv[:, :, 0:F],
                                in0=wv[:, :, 0:F],
                                in1=wv[:, :, (W - 1) * F:W * F],
                                op=mybir.AluOpType.add,
                            )
                            W -= 1
                        else:
                            hw = W // 2
                            outv = agg_v if hw == 1 else wv[:, :, 0:hw * F]
                            nc.vector.tensor_tensor(
                                out=outv,
                                in0=wv[:, :, 0:hw * F],
                                in1=wv[:, :, hw * F:W * F],
                                op=mybir.AluOpType.add,
                            )
                            W = hw

                if is_ov and ov_Kb > 0 and ov_max_entries > 1:
                    base = ov_single // P
                    for e in range(1, ov_max_entries):
                        b0 = base + e * ov_Kb
                        nc.vector.tensor_tensor(
                            out=agg_c[:, base * F:(base + ov_Kb) * F],
                            in0=agg_c[:, base * F:(base + ov_Kb) * F],
                            in1=agg_c[:, b0 * F:(b0 + ov_Kb) * F],
                            op=mybir.AluOpType.add,
                        )

                # h = relu(agg + xroot) -> bf16 (relu on ACT to offload DVE)
                nc.vector.tensor_tensor(
                    out=agg_c[:], in0=agg_c[:], in1=xr_v, op=mybir.AluOpType.add
                )
                h_c = pool.tile([P, nb * F], mybir.dt.bfloat16, tag=f"h{ki}")
                nc.scalar.activation(
                    out=h_c[:], in_=agg_c[:], func=mybir.ActivationFunctionType.Relu
                )
                for b in range(nb):
                    nc.tensor.matmul(
                        out=pt[:],
                        lhsT=cb_v[:, b * P:(b + 1) * P],
                        rhs=h_c[:, b * F:(b + 1) * F],
                        start=(bglobal == 0),
                        stop=(bglobal == Eb - 1),
                    )
                    bglobal += 1

            res = pool.tile([P, F], mybir.dt.float32)
            nc.vector.tensor_copy(out=res[:], in_=pt[:])
            nc.sync.dma_start(out=p_out[:], in_=res[:])
    return nc


# ----------------------------------------------------------------------------
# main entry
# ----------------------------------------------------------------------------
def kernel(**inputs) -> np.ndarray:
    x = np.asarray(inputs["x"], dtype=np.float32)
    edge_index = np.asarray(inputs["edge_index"])
    batch = np.asarray(inputs["batch"], dtype=np.int64)
    W1_rel = np.asarray(inputs["W1_rel"], dtype=np.float32)
    W1_root = np.asarray(inputs["W1_root"], dtype=np.float32)
    b1 = np.asarray(inputs["b1"], dtype=np.float32)
    W2_rel = np.asarray(inputs["W2_rel"], dtype=np.float64)
    W2_root = np.asarray(inputs["W2_root"], dtype=np.float64)
    b2 = np.asarray(inputs["b2"], dtype=np.float64)
    Wlin = np.asarray(inputs["Wlin"], dtype=np.float64)
    blin = np.asarray(inputs["blin"], dtype=np.float64)

    n_nodes, nfeat = x.shape
    shard = n_nodes // N_CORES
    nodes_pad = _ceil(shard, P) * P
    mchunks = nodes_pad // P

    common, cores = _build_structure(edge_index, n_nodes)

    # ---- launch 1 ----
    nc1 = _build_launch1(nodes_pad, nfeat)
    _split_sync_waits(nc1)
    # W = W_hi + W_lo (both bf16) recovers f32 weight precision via two
    # PSUM-accumulated matmuls.
    Wcat = np.concatenate([W1_rel, W1_root], axis=1)  # [64, 32]
    W_hi = Wcat.astype(ml_dtypes.bfloat16)
    W_lo = (Wcat - W_hi.astype(np.float32)).astype(ml_dtypes.bfloat16)
    W1 = np.concatenate([W_hi, W_lo], axis=1)  # [64, 64] bf16
    in_maps1 = []
    for c in range(N_CORES):
        xs = x[c * shard:(c + 1) * shard]
        xT = np.zeros((nfeat, nodes_pad), ml_dtypes.bfloat16)
        xT[:, :shard] = xs.T.astype(ml_dtypes.bfloat16)
        in_maps1.append({"xT": xT, "W1": W1})
    res1 = run_bass_kernel_spmd(nc1, in_maps1, list(range(N_CORES)))

    # decode per-core outputs into full-node tables
    xw_full = np.zeros((n_nodes, F), np.float32)
    xroot_full = np.zeros((n_nodes, F), np.float32)
    for c in range(N_CORES):
        dec = (
            np.asarray(res1.results[c]["xw_out"])
            .astype(np.float32)
            .reshape(P, mchunks, 32)
            .transpose(1, 0, 2)
            .reshape(nodes_pad, 32)
        )
        xw_full[c * shard:(c + 1) * shard] = dec[:shard, :F]
        xroot_full[c * shard:(c + 1) * shard] = dec[:shard, F:]

    # ---- host: expand messages + build cb tables ----
    E = common["E"]
    Eb = E // P
    gdst = batch[np.asarray(edge_index[1], dtype=np.int64)]
    src64 = np.asarray(edge_index[0], dtype=np.int64)
    Cmat = np.zeros((n_nodes, N_GRAPHS), np.float32)
    np.add.at(Cmat, (src64, gdst), 1.0)

    in_maps2 = []
    cw, cn = common["class_widths"], common["class_nblocks"]
    for st in cores:
        ss = st["slot_src"]
        m = np.where(ss[:, :, None] >= 0, xw_full[np.maximum(ss, 0)], 0.0)
        msgs = np.ascontiguousarray(
            m.reshape(P, -1).astype(ml_dtypes.bfloat16)
        )
        en = st["entry_node"]
        prim = st["primary"] & (en >= 0)
        # xroot' = x@W1_root + b1 (bias folded here on host)
        xr = np.where(
            prim[:, None], xroot_full[np.maximum(en, 0)] + b1, 0.0
        ).astype(np.float32)
        xroot_dev = np.ascontiguousarray(
            xr.reshape(Eb, P, F).transpose(1, 0, 2).reshape(P, Eb * F)
        ).astype(ml_dtypes.bfloat16)
        cb = np.zeros((E, 2 * N_GRAPHS), np.float32)
        cb[prim, :N_GRAPHS] = Cmat[en[prim]]
        cb[prim, N_GRAPHS + batch[en[prim]]] = 1.0
        cb_dev = np.ascontiguousarray(
            cb.reshape(Eb, P, 2 * N_GRAPHS)
            .transpose(1, 0, 2)
            .reshape(P, Eb * 2 * N_GRAPHS)
            .astype(ml_dtypes.float8_e4m3fn)
        )
        # pack blob per plan chunk: [msgs | cb | xroot] bytes, plan order
        plan = _class_plan(cw, cn)
        parts = []
        for (col0, eb0, w, nb, is_ov) in plan:
            parts.append(msgs[:, col0 * F:(col0 + nb * w) * F].view(np.uint8))
            parts.append(cb_dev[:, eb0 * P:(eb0 + nb) * P].view(np.uint8))
            parts.append(xroot_dev[:, eb0 * F:(eb0 + nb) * F].view(np.uint8))
        blob = np.ascontiguousarray(np.concatenate(parts, axis=1))
        in_maps2.append({"blob": blob})

    # ---- launch 2 ----
    nc2 = _build_launch2(
        common["class_widths"], common["class_nblocks"], common["ov_single"],
        common["ov_Kb"], common["ov_max_entries"],
    )
    _split_sync_waits(nc2)
    res2 = run_bass_kernel_spmd(nc2, in_maps2, list(range(N_CORES)))

    # ---- host finish ----
    P_total = np.zeros((2 * N_GRAPHS, F), np.float64)
    for c in range(N_CORES):
        P_total += np.asarray(res2.results[c]["p_out"]).astype(np.float64)
    v2 = (W2_rel @ Wlin)[:, 0]
    vr = (W2_root @ Wlin)[:, 0]
    cnt = np.bincount(batch, minlength=N_GRAPHS).astype(np.float64)[:N_GRAPHS]
    cnt = np.maximum(cnt, 1.0)
    out = (P_total[:N_GRAPHS] @ v2 + P_total[N_GRAPHS:] @ vr) / cnt
    out = out + (b2 @ Wlin)[0] + blin[0]
    return out.astype(np.float32)


# revision 21
# speedup vs baseline: 1.7846x; 1.0271x over previous
"""Trainium2 Bass kernel for the 2-layer GraphConv + mean-pool network.

Self-contained: kernel(**inputs) -> np.ndarray [N_GRAPHS] float32.

Strategy (8 NeuronCores, SPMD, 2 launches):
  Launch 1: per-core node shard, compute x @ [W1_rel | W1_root] on the
    tensor engine (bf16 x; f32 weight precision recovered via a
    W = W_hi + W_lo bf16 residual split, two PSUM-accumulated matmuls).
    This exploits segment_sum(x[src]) @ W == segment_sum((x@W)[src]) to cut
    edge traffic 4x. b1 is folded into the xroot staging on the host.
  Host: expand (x@W1_rel)[src] per edge into a degree-class-sorted,
    partition-aligned layout (pure index/permutation work), replicating the
    halo exchange. Nodes are sharded by contiguous dst ranges.
  Launch 2: per-core, stream edge messages sequentially; segmented reduction
    over each node's incident edges via log-halving strided vector adds
    (bf16 in, f32 accumulate); h = relu(agg + x@W1_root + b1); then a single
    PSUM-accumulated matmul P = [C|B]^T @ h where C[j,g] = #out-edges of node
    j landing in graph g and B = one-hot(batch). This collapses the entire
    second GraphConv layer + global mean pool into one matmul because the
    final output only needs per-graph sums (linearity of layer 2).
  Host: out[g] = (P[g]@(W2_rel@Wlin) + P[64+g]@(W2_root@Wlin)) / count_g
                 + b2@Wlin + blin   (a 64-element finish).
"""
import sys

if "/opt/trn_rl_repo" not in sys.path:
    sys.path.insert(0, "/opt/trn_rl_repo")

import numpy as np
import ml_dtypes

import concourse.bass as bass
import concourse.mybir as mybir
import concourse.tile as tile
from concourse.vector_clock import ScopedClock
from concourse.bass_utils import run_bass_kernel_spmd

N_CORES = 8
P = 128
F = 16
WMAX = 16
N_GRAPHS = 64

# ----------------------------------------------------------------------------
# toolchain workarounds
# ----------------------------------------------------------------------------
_PATCHED = False


def _patch_tile():
    """Walrus (neuronxcc) rejects >~2 sync waits on one instruction; Tile's
    final drain can carry many. Emit them as separate nops instead."""
    global _PATCHED
    if _PATCHED:
        return
    _PATCHED = True

    def patched(self, tick_clock, wait_clock):
        nop = self.nc.sync.nop(nofuse=True)
        wait_clock.add_sem_waits(nop.ins, ScopedClock({None: tick_clock.global_clock}))
        si = nop.ins.sync_info
        if si is not None and si.on_wait and len(si.on_wait) > 1:
            waits = list(si.on_wait)
            si.on_wait = waits[:1]
            for w in waits[1:]:
                nop2 = self.nc.sync.nop(nofuse=True)
                si2 = nop2.ins.sync_info
                if si2 is None:
                    nop2.ins.sync_info = mybir.SyncInfo(on_wait=[w], on_update=[])
                else:
                    si2.on_wait = [w]
        self.nc.sync.drain()
        self.nc.all_engine_barrier()
        assert self.sems is not None
        popped = self.nc._tile_sem_poison_stack.pop()
        assert popped is self._sem_poison
        self.nc.clear_and_free_semaphores(list(self.sems.allocated().values()))
        self.nc.all_engine_barrier()

    tile.TileContext._drain_and_barrier = patched


def _split_sync_waits(nc, max_waits=1):
    """Move excess per-instruction sync waits onto injected NoOps."""
    import bass_rust

    ctr = 0
    for fn in nc.m.functions:
        for bb in fn.blocks:
            insts = list(bb.instructions)
            out = []
            changed = False
            for inst in insts:
                si = getattr(inst, "sync_info", None)
                if si is not None and si.on_wait and len(si.on_wait) > max_waits:
                    waits = list(si.on_wait)
                    for w in waits[:-max_waits]:
                        nop = bass_rust.InstNoOp(name=f"wsplit-{ctr}", ins=[], outs=[])
                        ctr += 1
                        nop.engine = inst.engine
                        nop.sync_info = mybir.SyncInfo(on_wait=[w], on_update=[])
                        out.append(nop)
                    si.on_wait = waits[-max_waits:]
                    changed = True
                out.append(inst)
            if changed:
                bb.instructions = out
    return nc


_patch_tile()


def _ceil(a, b):
    return -(-a // b)


# ----------------------------------------------------------------------------
# host-side structure
# ----------------------------------------------------------------------------
def _build_structure(edge_index, n_nodes):
    src = np.asarray(edge_index[0], dtype=np.int64)
    dst = np.asarray(edge_index[1], dtype=np.int64)
    shard = n_nodes // N_CORES

    per_core = []
    for c in range(N_CORES):
        lo = c * shard
        esel = (dst >= lo) & (dst < lo + shard)
        esrc = src[esel].astype(np.int32)
        edst = (dst[esel] - lo).astype(np.int32)
        deg = np.bincount(edst, minlength=shard)
        order = np.argsort(edst, kind="stable")
        esrc_sorted = esrc[order]
        starts = np.zeros(shard + 1, dtype=np.int64)
        np.cumsum(deg, out=starts[1:])
        d_primary = np.minimum(deg, WMAX)
        w_primary = np.maximum(2, 2 * ((d_primary + 1) // 2))
        n_entries = np.maximum(1, _ceil_arr(deg, WMAX))
        per_core.append(
            dict(
                deg=deg, starts=starts, esrc=esrc_sorted, lo=lo,
                w_primary=w_primary, n_entries=n_entries,
                ov_nodes=np.where(n_entries > 1)[0],
            )
        )

    widths = list(range(2, WMAX + 1, 2))
    singles_blocks = 0
    Kb = 0
    max_entries = 1
    for st in per_core:
        max_entries = max(max_entries, int(st["n_entries"].max()))
        Kb = max(Kb, _ceil(len(st["ov_nodes"]), P))
        n16 = int(((st["w_primary"] == WMAX) & (st["n_entries"] == 1)).sum())
        singles_blocks = max(singles_blocks, _ceil(n16, P))
    nb_common = []
    for w in widths[:-1]:
        mx = 0
        for st in per_core:
            n = int(((st["w_primary"] == w) & (st["n_entries"] == 1)).sum())
            mx = max(mx, _ceil(n, P))
        nb_common.append(mx)
    nb_common.append(singles_blocks + Kb * max_entries)

    common = dict(
        class_widths=widths,
        class_nblocks=nb_common,
        ov_single=singles_blocks * P,
        ov_Kb=Kb,
        ov_max_entries=max_entries,
        E=sum(nb * P for nb in nb_common),
        TOT=sum(nb * w for nb, w in zip(nb_common, widths)),
    )

    cores = []
    for st in per_core:
        deg, starts, esrc_sorted, lo = st["deg"], st["starts"], st["esrc"], st["lo"]
        w_primary, n_entries = st["w_primary"], st["n_entries"]
        entry_node = []
        entry_eidx = []
        for w, nb in zip(widths, nb_common):
            if w < WMAX:
                nodes_w = np.where((w_primary == w) & (n_entries == 1))[0]
                lst = [(int(n), 0) for n in nodes_w]
            else:
                nodes_w = np.where((w_primary == WMAX) & (n_entries == 1))[0]
                lst = [(int(n), 0) for n in nodes_w]
                lst += [(-1, 0)] * (singles_blocks * P - len(lst))
                for e in range(max_entries):
                    sub = [
                        (int(n), e) if e < n_entries[n] else (-1, 0)
                        for n in st["ov_nodes"]
                    ]
                    sub += [(-1, 0)] * (Kb * P - len(sub))
                    lst += sub
            lst += [(-1, 0)] * (nb * P - len(lst))
            entry_node.extend(n for n, _ in lst)
            entry_eidx.extend(e for _, e in lst)
        entry_node = np.array(entry_node, dtype=np.int64)
        entry_eidx = np.array(entry_eidx, dtype=np.int64)

        TOT = common["TOT"]
        slot_src = np.full((P, TOT), -1, dtype=np.int32)
        col = 0
        epos = 0
        for w, nb in zip(widths, nb_common):
            idx = np.arange(nb * P)
            nodes = entry_node[epos + idx]
            eidxs = entry_eidx[epos + idx]
            pp = idx % P
            bb = idx // P
            valid = nodes >= 0
            nval = nodes[valid].astype(np.int64)
            for k in range(w):
                epos_k = eidxs[valid] * WMAX + k
                has = epos_k < deg[nval]
                rows = pp[valid][has]
                cols = col + bb[valid][has] * w + k
                slot_src[rows, cols] = esrc_sorted[starts[nval[has]] + epos_k[has]]
            epos += nb * P
            col += nb * w

        primary = (entry_eidx == 0) & (entry_node >= 0)
        entry_node_g = np.where(entry_node >= 0, entry_node + lo, -1).astype(np.int32)
        cores.append(dict(slot_src=slot_src, entry_node=entry_node_g, primary=primary))
    return common, cores


def _ceil_arr(a, b):
    return -(-a // b)


# ----------------------------------------------------------------------------
# bass kernel builders
# ----------------------------------------------------------------------------
def _build_launch1(nodes_pad, nfeat=64):
    """xw_out[p, m*32+f] = (x @ [W1_rel|W1_root])[m*128+p, f]

    x is bf16; full f32 weight precision is recovered by splitting
    W = W_hi + W_lo (both bf16) and accumulating two matmuls in PSUM.
    xT is loaded in 3 column chunks (DMA count dominates fixed cost);
    the output is staged into one SBUF tile and written with one DMA."""
    mchunks = nodes_pad // P
    nc = bass.Bass()
    xT = nc.dram_tensor("xT", [nfeat, nodes_pad], mybir.dt.bfloat16, kind="ExternalInput")
    W1 = nc.dram_tensor("W1", [nfeat, 64], mybir.dt.bfloat16, kind="ExternalInput")
    xw_out = nc.dram_tensor(
        "xw_out", [P, mchunks * 32], mybir.dt.bfloat16, kind="ExternalOutput"
    )
    NDMA = 3
    per = _ceil(mchunks, NDMA)
    GROUP = 16  # chunks per psum bank (16*32 = 512 f32)

    with tile.TileContext(nc) as tc:
        with (
            tc.tile_pool(name="sbuf", bufs=1) as pool,
            tc.tile_pool(name="xtp", bufs=3) as xtp,
            tc.tile_pool(name="psum", bufs=4, space="PSUM") as psum_pool,
        ):
            W1_sb = pool.tile([nfeat, 64], mybir.dt.bfloat16)
            out_sb = pool.tile([P, mchunks * 32], mybir.dt.bfloat16)
            for d in range(NDMA):
                c0 = d * per
                c1 = min(c0 + per, mchunks)
                xt_g = xtp.tile([nfeat, (c1 - c0) * P], mybir.dt.bfloat16, tag="xt")
                nc.sync.dma_start(out=xt_g[:], in_=xT[:, c0 * P:c1 * P])
                if d == 0:
                    # issue after chunk 0 so the big transfer leads; W1 is
                    # tiny and rides behind it
                    nc.sync.dma_start(out=W1_sb[:], in_=W1[:])
                for g0 in range(c0, c1, GROUP):
                    g1 = min(g0 + GROUP, c1)
                    pt = psum_pool.tile([P, 512], mybir.dt.float32, tag="ps")
                    for m in range(g0, g1):
                        sl = pt[:, (m - g0) * 32:(m - g0 + 1) * 32]
                        lhs = xt_g[:, (m - c0) * P:(m - c0 + 1) * P]
                        nc.tensor.matmul(out=sl, lhsT=lhs, rhs=W1_sb[:, 0:32],
                                         start=True, stop=False)
                        nc.tensor.matmul(out=sl, lhsT=lhs, rhs=W1_sb[:, 32:64],
                                         start=False, stop=True)
                    n = (g1 - g0) * 32
                    # NOTE: ACT (scalar) reading PSUM crashes the device on
                    # this toolchain -- keep this copy on DVE.
                    nc.vector.tensor_copy(
                        out=out_sb[:, g0 * 32:g0 * 32 + n], in_=pt[:, :n]
                    )
                # stream this chunk's output while the next chunk computes
                nc.sync.dma_start(
                    out=xw_out[:, c0 * 32:c1 * 32], in_=out_sb[:, c0 * 32:c1 * 32]
                )
    return nc


def _class_plan(class_widths, class_nblocks, max_nb=10):
    """Emission plan for launch 2: split big classes into block-chunks and
    order them largest-first so the pipeline tail lands on small chunks.
    Each entry: (col0, eb0, w, nb) with offsets in the ORIGINAL layout.
    The last original class (the WMAX/overflow class) is never split and is
    flagged so the overflow combine runs inside it."""
    ncls = len(class_widths)
    chunks = []
    col = 0
    eb0 = 0
    for ci, (w, nb) in enumerate(zip(class_widths, class_nblocks)):
        if nb == 0:
            col += nb * w
            eb0 += nb
            continue
        is_ov = ci == ncls - 1
        if is_ov or nb <= max_nb:
            parts = [nb]
        else:
            k = _ceil(nb, max_nb)
            base = nb // k
            rem = nb % k
            parts = [base + (1 if i < rem else 0) for i in range(k)]
        o_col, o_eb = col, eb0
        for pnb in parts:
            chunks.append((o_col, o_eb, w, pnb, is_ov))
            o_col += pnb * w
            o_eb += pnb
        col += nb * w
        eb0 += nb
    # largest data first; overflow chunk pinned to the end of the big ones?
    # simple: sort by descending message bytes, ov chunk sorts with them.
    chunks.sort(key=lambda c: -(c[3] * c[2]))
    return chunks


def _build_launch2(class_widths, class_nblocks, ov_single, ov_Kb, ov_max_entries,
                   max_nb=10):
    """Per-core launch 2. One packed uint8 "blob" input; one DMA per plan
    chunk covering [msgs bf16 | cb f8 | xroot bf16] segments (bitcast views).
    Chunks are emitted largest-first (see _class_plan)."""
    Eb = sum(class_nblocks)
    plan = _class_plan(class_widths, class_nblocks, max_nb)
    # blob layout: per plan chunk, [msgs | cb | xroot] bytes, in plan order
    seg = []
    off = 0
    for (col0, eb0, w, nb, is_ov) in plan:
        bm = nb * w * F * 2
        bc = nb * P
        bx = nb * F * 2
        seg.append((off, off + bm, off + bm + bc, bm + bc + bx))
        off += bm + bc + bx
    total_bytes = off

    nc = bass.Bass()
    blob_d = nc.dram_tensor("blob", [P, total_bytes], mybir.dt.uint8, kind="ExternalInput")
    p_out = nc.dram_tensor("p_out", [P, F], mybir.dt.float32, kind="ExternalOutput")

    with tile.TileContext(nc) as tc:
        with (
            tc.tile_pool(name="sbuf", bufs=1) as pool,
            tc.tile_pool(name="psum", bufs=1, space="PSUM") as psum_pool,
        ):
            pt = psum_pool.tile([P, F], mybir.dt.float32)
            bglobal = 0
            for ki, ((col0, eb0, w, nb, is_ov), (o_m, o_c, o_x, nbytes)) in enumerate(
                zip(plan, seg)
            ):
                bt = pool.tile([P, nbytes], mybir.dt.uint8, tag=f"b{ki}")
                nc.sync.dma_start(out=bt[:], in_=blob_d[:, o_m:o_m + nbytes])
                mv = bt[:, 0:nb * w * F * 2].bitcast(mybir.dt.bfloat16).rearrange(
                    "p (n q) -> p n q", q=w * F
                )
                cb_v = bt[:, o_c - o_m:o_c - o_m + nb * P].bitcast(mybir.dt.float8e4)
                xr_v = bt[:, o_x - o_m:o_x - o_m + nb * F * 2].bitcast(mybir.dt.bfloat16)

                agg_c = pool.tile([P, nb * F], mybir.dt.bfloat16, tag=f"a{ki}")
                agg_v = agg_c[:].rearrange("p (n q) -> p n q", q=F)
                if w == 2:
                    nc.vector.tensor_tensor(
                        out=agg_v, in0=mv[:, :, 0:F], in1=mv[:, :, F:2 * F],
                        op=mybir.AluOpType.add,
                    )
                else:
                    half = w // 2
                    wt = pool.tile([P, nb * half * F], mybir.dt.bfloat16, tag=f"w{ki}")
                    nc.vector.tensor_tensor(
                        out=wt[:].rearrange("p (n q) -> p n q", q=half * F),
                        in0=mv[:, :, 0:half * F],
                        in1=mv[:, :, half * F:w * F],
                        op=mybir.AluOpType.add,
                    )
                    W = half
                    while W > 1:
                        wv = wt[:].rearrange("p (n q) -> p n q", q=half * F)
                        if W % 2 == 1:
                            nc.vector.tensor_tensor(
                                out=wv[:, :, 0:F],
                                in0=wv[:, :, 0:F],
                                in1=wv[:, :, (W - 1) * F:W * F],
                                op=mybir.AluOpType.add,
                            )
                            W -= 1
                        else:
                            hw = W // 2
                            outv = agg_v if hw == 1 else wv[:, :, 0:hw * F]
                            nc.vector.tensor_tensor(
                                out=outv,
                                in0=wv[:, :, 0:hw * F],
                                in1=wv[:, :, hw * F:W * F],
                                op=mybir.AluOpType.add,
                            )
                            W = hw

                if is_ov and ov_Kb > 0 and ov_max_entries > 1:
                    base = ov_single // P
                    for e in range(1, ov_max_entries):
                        b0 = base + e * ov_Kb
                        nc.vector.tensor_tensor(
                            out=agg_c[:, base * F:(base + ov_Kb) * F],
                            in0=agg_c[:, base * F:(base + ov_Kb) * F],
                            in1=agg_c[:, b0 * F:(b0 + ov_Kb) * F],
                            op=mybir.AluOpType.add,
                        )

                # h = relu(agg + xroot) -> bf16 (relu on ACT to offload DVE)
                nc.vector.tensor_tensor(
                    out=agg_c[:], in0=agg_c[:], in1=xr_v, op=mybir.AluOpType.add
                )
                h_c = pool.tile([P, nb * F], mybir.dt.bfloat16, tag=f"h{ki}")
                nc.scalar.activation(
                    out=h_c[:], in_=agg_c[:], func=mybir.ActivationFunctionType.Relu
                )
                for b in range(nb):
                    nc.tensor.matmul(
                        out=pt[:],
                        lhsT=cb_v[:, b * P:(b + 1) * P],
                        rhs=h_c[:, b * F:(b + 1) * F],
                        start=(bglobal == 0),
                        stop=(bglobal == Eb - 1),
                    )
                    bglobal += 1

            res = pool.tile([P, F], mybir.dt.float32)
            nc.vector.tensor_copy(out=res[:], in_=pt[:])
            nc.sync.dma_start(out=p_out[:], in_=res[:])
    return nc


# ----------------------------------------------------------------------------
# main entry
# ----------------------------------------------------------------------------
def kernel(**inputs) -> np.ndarray:
    x = np.asarray(inputs["x"], dtype=np.float32)
    edge_index = np.asarray(inputs["edge_index"])
    batch = np.asarray(inputs["batch"], dtype=np.int64)
    W1_rel = np.asarray(inputs["W1_rel"], dtype=np.float32)
    W1_root = np.asarray(inputs["W1_root"], dtype=np.float32)
    b1 = np.asarray(inputs["b1"], dtype=np.float32)
    W2_rel = np.asarray(inputs["W2_rel"], dtype=np.float64)
    W2_root = np.asarray(inputs["W2_root"], dtype=np.float64)
    b2 = np.asarray(inputs["b2"], dtype=np.float64)
    Wlin = np.asarray(inputs["Wlin"], dtype=np.float64)
    blin = np.asarray(inputs["blin"], dtype=np.float64)

    n_nodes, nfeat = x.shape
    shard = n_nodes // N_CORES
    nodes_pad = _ceil(shard, P) * P
    mchunks = nodes_pad // P

    common, cores = _build_structure(edge_index, n_nodes)

    # ---- launch 1 ----
    nc1 = _build_launch1(nodes_pad, nfeat)
    _split_sync_waits(nc1)
    # W = W_hi + W_lo (both bf16) recovers f32 weight precision via two
    # PSUM-accumulated matmuls.
    Wcat = np.concatenate([W1_rel, W1_root], axis=1)  # [64, 32]
    W_hi = Wcat.astype(ml_dtypes.bfloat16)
    W_lo = (Wcat - W_hi.astype(np.float32)).astype(ml_dtypes.bfloat16)
    W1 = np.concatenate([W_hi, W_lo], axis=1)  # [64, 64] bf16
    in_maps1 = []
    for c in range(N_CORES):
        xs = x[c * shard:(c + 1) * shard]
        xT = np.zeros((nfeat, nodes_pad), ml_dtypes.bfloat16)
        xT[:, :shard] = xs.T.astype(ml_dtypes.bfloat16)
        in_maps1.append({"xT": xT, "W1": W1})
    res1 = run_bass_kernel_spmd(nc1, in_maps1, list(range(N_CORES)))

    # decode per-core outputs into full-node tables
    xw_full = np.zeros((n_nodes, F), np.float32)
    xroot_full = np.zeros((n_nodes, F), np.float32)
    for c in range(N_CORES):
        dec = (
            np.asarray(res1.results[c]["xw_out"])
            .astype(np.float32)
            .reshape(P, mchunks, 32)
            .transpose(1, 0, 2)
            .reshape(nodes_pad, 32)
        )
        xw_full[c * shard:(c + 1) * shard] = dec[:shard, :F]
        xroot_full[c * shard:(c + 1) * shard] = dec[:shard, F:]

    # ---- host: expand messages + build cb tables ----
    E = common["E"]
    Eb = E // P
    gdst = batch[np.asarray(edge_index[1], dtype=np.int64)]
    src64 = np.asarray(edge_index[0], dtype=np.int64)
    Cmat = np.zeros((n_nodes, N_GRAPHS), np.float32)
    np.add.at(Cmat, (src64, gdst), 1.0)

    in_maps2 = []
    cw, cn = common["class_widths"], common["class_nblocks"]
    for st in cores:
        ss = st["slot_src"]
        m = np.where(ss[:, :, None] >= 0, xw_full[np.maximum(ss, 0)], 0.0)
        msgs = np.ascontiguousarray(
            m.reshape(P, -1).astype(ml_dtypes.bfloat16)
        )
        en = st["entry_node"]
        prim = st["primary"] & (en >= 0)
        # xroot' = x@W1_root + b1 (bias folded here on host)
        xr = np.where(
            prim[:, None], xroot_full[np.maximum(en, 0)] + b1, 0.0
        ).astype(np.float32)
        xroot_dev = np.ascontiguousarray(
            xr.reshape(Eb, P, F).transpose(1, 0, 2).reshape(P, Eb * F)
        ).astype(ml_dtypes.bfloat16)
        cb = np.zeros((E, 2 * N_GRAPHS), np.float32)
        cb[prim, :N_GRAPHS] = Cmat[en[prim]]
        cb[prim, N_GRAPHS + batch[en[prim]]] = 1.0
        cb_dev = np.ascontiguousarray(
            cb.reshape(Eb, P, 2 * N_GRAPHS)
            .transpose(1, 0, 2)
            .reshape(P, Eb * 2 * N_GRAPHS)
            .astype(ml_dtypes.float8_e4m3fn)
        )
        # pack blob per plan chunk: [msgs | cb | xroot] bytes, plan order
        plan = _class_plan(cw, cn)
        parts = []
        for (col0, eb0, w, nb, is_ov) in plan:
            parts.append(msgs[:, col0 * F:(col0 + nb * w) * F].view(np.uint8))
            parts.append(cb_dev[:, eb0 * P:(eb0 + nb) * P].view(np.uint8))
            parts.append(xroot_dev[:, eb0 * F:(eb0 + nb) * F].view(np.uint8))
        blob = np.ascontiguousarray(np.concatenate(parts, axis=1))
        in_maps2.append({"blob": blob})

    # ---- launch 2 ----
    nc2 = _build_launch2(
        common["class_widths"], common["class_nblocks"], common["ov_single"],
        common["ov_Kb"], common["ov_max_entries"],
    )
    _split_sync_waits(nc2)
    res2 = run_bass_kernel_spmd(nc2, in_maps2, list(range(N_CORES)))

    # ---- host finish ----
    P_total = np.zeros((2 * N_GRAPHS, F), np.float64)
    for c in range(N_CORES):
        P_total += np.asarray(res2.results[c]["p_out"]).astype(np.float64)
    v2 = (W2_rel @ Wlin)[:, 0]
    vr = (W2_root @ Wlin)[:, 0]
    cnt = np.bincount(batch, minlength=N_GRAPHS).astype(np.float64)[:N_GRAPHS]
    cnt = np.maximum(cnt, 1.0)
    out = (P_total[:N_GRAPHS] @ v2 + P_total[N_GRAPHS:] @ vr) / cnt
    out = out + (b2 @ Wlin)[0] + blin[0]
    return out.astype(np.float32)
